# revision 1
# baseline (speedup 1.0000x reference)
"""Trainium2 Bass kernel for nn_MAPLoss (smooth-AP loss, N=512, D=256, K=0.001).

v5: positives-only evaluation with tight pair packing. The loss reads
prec[i] only at positive (query, item) pairs (~3600 of 512*511), so each
core evaluates just its ~450 pairs, bin-packed row-atomically into
[128-partition x 512] ACT blocks (typically 4 per core):
  - rrep[p, :] = ranking row of row(p) (replication matmul on PE),
  - rg[p] = R[row(p), sel[p]] via fused iota==sel multiply-accumulate (DVE),
  - den[p] = sum_j sigmoid(1000*(r_j - rg[p])): one ACT op per block with
    accum_out producing the row sums for free,
  - acc[p] = sum over positive-positive pairs only, reconstructed from the
    gathered rg values with block-diagonal selector matmuls ([128, 16]
    per block) and one batched sigmoid,
  - epilogue: prec = (acc+0.5)/(den+0.5-sigmoid(1000*(1-rg))), then one
    weighted global sum with host-folded weights w = 1/npos at real pairs.
Self/diagonal corrections account for the reference's 511-item sums.
Host passes only index metadata (pair slots, selector/one-hot matrices,
weights) derived from `target`; all FLOPs (normalize, R = qn qn^T,
sigmoids, reductions, division) run on device in fp32.
Each core returns its partial numerator; the host sums and finishes
1 - num/cnt (cnt = number of valid rows, exact integer metadata).
"""

import numpy as np
from contextlib import ExitStack

N = 512
D = 256
NCORES = 8
RPC = N // NCORES   # rows per core = 64
SLOTS = 16          # max positives per row (max npos observed is 13)
KINV = 1000.0       # 1/K


def _build_program(nblk):
    import concourse.bacc as bacc
    import concourse.tile as tile
    import concourse.mybir as mybir

    fp32 = mybir.dt.float32
    ALU = mybir.AluOpType
    ACT = mybir.ActivationFunctionType
    AX = mybir.AxisListType

    nc = bacc.Bacc("TRN2", target_bir_lowering=False, debug=False,
                   num_devices=NCORES)
    q_dram = nc.dram_tensor("q", [N, D], fp32, kind="ExternalInput").ap()
    qt_dram = nc.dram_tensor("qt", [D, N], fp32, kind="ExternalInput").ap()
    sel_dram = nc.dram_tensor("sel", [128, nblk], fp32, kind="ExternalInput").ap()
    w_dram = nc.dram_tensor("w", [128, nblk], fp32, kind="ExternalInput").ap()
    maskg_dram = nc.dram_tensor("maskg", [128, SLOTS * nblk], fp32,
                                kind="ExternalInput").ap()
    rep_dram = nc.dram_tensor("rep", [RPC, 128 * nblk], fp32,
                              kind="ExternalInput").ap()
    bdgs_dram = nc.dram_tensor("bdgs", [128, 128 * nblk], fp32,
                               kind="ExternalInput").ap()
    ibs_dram = nc.dram_tensor("ibs", [128, SLOTS * nblk], fp32,
                              kind="ExternalInput").ap()
    iota_dram = nc.dram_tensor("iota", [128, N], fp32, kind="ExternalInput").ap()
    out_dram = nc.dram_tensor("out", [1, 1], fp32, kind="ExternalOutput").ap()

    NRC = N // 128  # 4 row chunks
    NDC = D // 128  # 2 dim chunks

    with tile.TileContext(nc) as tc, ExitStack() as ctx:
        const = ctx.enter_context(tc.tile_pool(name="const", bufs=1))
        setup = ctx.enter_context(tc.tile_pool(name="setup", bufs=2))
        setup_ctx = ctx.enter_context(ExitStack())
        spsum = setup_ctx.enter_context(
            tc.tile_pool(name="spsum", bufs=1, space="PSUM"))
        persist = ctx.enter_context(tc.tile_pool(name="persist", bufs=1))

        # --- constants / metadata loads (spread across DMA rings) ---
        ones_col = const.tile([128, 1], fp32, tag="ones_col")
        nc.gpsimd.memset(ones_col[:], 1.0)
        ones_row = const.tile([1, 128], fp32, tag="ones_row")
        nc.gpsimd.memset(ones_row[:], 1.0)
        ib_ones = const.tile([128, SLOTS], fp32, tag="ib_ones")
        nc.gpsimd.memset(ib_ones[:], 1.0)
        k1000 = const.tile([128, 1], fp32, tag="k1000")
        nc.gpsimd.memset(k1000[:], KINV)
        iota_f = const.tile([128, N], fp32, tag="iota_f")
        nc.scalar.dma_start(iota_f[:], iota_dram)

        rep = persist.tile([RPC, 128 * nblk], fp32, tag="rep")
        nc.gpsimd.dma_start(rep[:], rep_dram)
        bdgs = persist.tile([128, 128 * nblk], fp32, tag="bdgs")
        nc.gpsimd.dma_start(bdgs[:], bdgs_dram)
        sel = persist.tile([128, nblk], fp32, tag="sel")
        nc.scalar.dma_start(sel[:], sel_dram)
        ibs = persist.tile([128, SLOTS * nblk], fp32, tag="ibs")
        nc.scalar.dma_start(ibs[:], ibs_dram)
        w_t = persist.tile([128, nblk], fp32, tag="w_t")
        nc.scalar.dma_start(w_t[:], w_dram)
        maskg = persist.tile([128, SLOTS * nblk], fp32, tag="maskg")
        nc.scalar.dma_start(maskg[:], maskg_dram)

        # --- q row-chunks (norms) + host-transposed qT chunks ---
        qT = [persist.tile([128, N], fp32, tag=f"qT{dc}", name=f"qT{dc}")
              for dc in range(NDC)]
        for dc in range(NDC):
            nc.sync.dma_start(qT[dc][:], qt_dram[128 * dc:128 * (dc + 1), :])

        inv_row = persist.tile([1, N], fp32, tag="inv_row")
        for rc in range(NRC):
            qc = setup.tile([128, D], fp32, tag="qload")
            nc.sync.dma_start(qc[:], q_dram[rc * 128:(rc + 1) * 128, :])
            sq = setup.tile([128, 1], fp32, tag="sq")
            scratch = setup.tile([128, D], fp32, tag="sqscratch")
            nc.scalar.activation(scratch[:], qc[:], ACT.Square, accum_out=sq[:])
            norm = setup.tile([128, 1], fp32, tag="norm")
            nc.scalar.activation(norm[:], sq[:], ACT.Sqrt)
            nc.vector.tensor_scalar_max(norm[:], norm[:], 1e-8)
            if rc == 0:
                inv = persist.tile([128, 1], fp32, tag="inv0", name="inv0")
                inv0 = inv
            else:
                inv = setup.tile([128, 1], fp32, tag="inv", bufs=3, name="inv")
            nc.vector.reciprocal(inv[:], norm[:])
            nc.sync.dma_start(inv_row[0:1, 128 * rc:128 * (rc + 1)], inv[:])

        # --- R = diag(inv) (q q^T) diag(inv) for rows 0..63 ---
        r_psum = spsum.tile([RPC, N], fp32, tag="rpsum")
        for dc in range(NDC):
            nc.tensor.matmul(r_psum[:], qT[dc][:, 0:RPC], qT[dc][:],
                             start=(dc == 0), stop=(dc == NDC - 1))
        ib_psum = spsum.tile([RPC, N], fp32, tag="ibpsum")
        nc.tensor.matmul(ib_psum[:], ones_row[0:1, 0:RPC], inv_row[:],
                         start=True, stop=True)
        R1 = setup.tile([RPC, N], fp32, tag="R1")
        nc.vector.tensor_scalar(R1[:], r_psum[:], inv0[0:RPC, :], None,
                                op0=ALU.mult)
        R = persist.tile([RPC, N], fp32, tag="R")
        nc.vector.tensor_mul(R[:], R1[:], ib_psum[:])

        # --- main: one [128, 512] ACT block per pair-bin ---
        rg_flat = persist.tile([128, nblk], fp32, tag="rg_flat")
        bias_flat = persist.tile([128, nblk], fp32, tag="bias_flat")
        den_flat = persist.tile([128, nblk], fp32, tag="den_flat")
        acc_flat = persist.tile([128, nblk], fp32, tag="acc_flat")
        setup_ctx.close()
        s_pool = ctx.enter_context(tc.tile_pool(name="s", bufs=3))
        rp_pool = ctx.enter_context(tc.tile_pool(name="rp", bufs=3, space="PSUM"))
        gp_pool = ctx.enter_context(tc.tile_pool(name="gp", bufs=2, space="PSUM"))

        for b in range(nblk):
            rrep = rp_pool.tile([128, N], fp32, tag="rrep")
            nc.tensor.matmul(rrep[:], rep[:, 128 * b:128 * (b + 1)], R[:],
                             start=True, stop=True)
            tmp = s_pool.tile([128, N], fp32, tag="gtmp")
            nc.vector.scalar_tensor_tensor(
                tmp[:], iota_f[:], sel[:, b:b + 1], rrep[:],
                op0=ALU.is_equal, op1=ALU.mult,
                accum_out=rg_flat[:, b:b + 1])
            nc.vector.tensor_scalar_mul(bias_flat[:, b:b + 1],
                                        rg_flat[:, b:b + 1], -KINV)
            sp = s_pool.tile([128, N], fp32, tag="sp")
            nc.scalar.activation(sp[:], rrep[:], ACT.Sigmoid,
                                 bias=bias_flat[:, b:b + 1], scale=KINV,
                                 accum_out=den_flat[:, b:b + 1])
            # acc from positive-positive pairs (gathered rg values):
            # RH[k,s'] = rg[k]*ibs_b[k,s']; G_b = bdgs_b^T @ RH gives
            # G[p,s'] = rg of slot s' of row(p); sigmoid(1000*(G - rg[p])).
            rh = s_pool.tile([128, SLOTS], fp32, tag="rh")
            nc.vector.tensor_scalar(rh[:], ibs[:, SLOTS * b:SLOTS * (b + 1)],
                                    rg_flat[:, b:b + 1], None, op0=ALU.mult)
            t2 = s_pool.tile([128, SLOTS], fp32, tag="t2")
            nc.vector.tensor_scalar(t2[:], ib_ones[:], rg_flat[:, b:b + 1],
                                    None, op0=ALU.mult)
            g_ps = gp_pool.tile([128, SLOTS], fp32, tag="g_ps")
            nc.tensor.matmul(g_ps[:], bdgs[:, 128 * b:128 * (b + 1)], rh[:],
                             start=True, stop=True)
            dd = s_pool.tile([128, SLOTS], fp32, tag="dd")
            nc.vector.tensor_sub(dd[:], g_ps[:], t2[:])
            ss = s_pool.tile([128, SLOTS], fp32, tag="ss")
            nc.scalar.activation(ss[:], dd[:], ACT.Sigmoid, scale=KINV)
            sacc = s_pool.tile([128, SLOTS], fp32, tag="sacc")
            nc.vector.scalar_tensor_tensor(
                sacc[:], ss[:], 1.0, maskg[:, SLOTS * b:SLOTS * (b + 1)],
                op0=ALU.mult, op1=ALU.mult,
                accum_out=acc_flat[:, b:b + 1])

        # --- epilogue: prec, weighted global sum ---
        ep = ctx.enter_context(tc.tile_pool(name="ep", bufs=1))
        s_colg = ep.tile([128, nblk], fp32, tag="s_colg")
        nc.scalar.activation(s_colg[:], bias_flat[:], ACT.Sigmoid,
                             bias=k1000[:], scale=1.0)
        den_adj = ep.tile([128, nblk], fp32, tag="den_adj")
        nc.vector.scalar_tensor_tensor(den_adj[:], den_flat[:], 0.5, s_colg[:],
                                       op0=ALU.add, op1=ALU.subtract)
        recip = ep.tile([128, nblk], fp32, tag="recip")
        nc.vector.reciprocal(recip[:], den_adj[:])
        acc_adj = ep.tile([128, nblk], fp32, tag="acc_adj")
        nc.vector.tensor_scalar_add(acc_adj[:], acc_flat[:], 0.5)
        prec = ep.tile([128, nblk], fp32, tag="prec")
        nc.vector.tensor_mul(prec[:], acc_adj[:], recip[:])
        pw = ep.tile([128, nblk], fp32, tag="pw")
        nc.vector.tensor_mul(pw[:], prec[:], w_t[:])
        nsum = ep.tile([128, 1], fp32, tag="nsum")
        nc.vector.tensor_reduce(nsum[:], pw[:], axis=AX.X, op=ALU.add)
        red = gp_pool.tile([1, 1], fp32, tag="red", bufs=1)
        nc.tensor.matmul(red[:], nsum[:], ones_col[:], start=True, stop=True)
        out_sb = ep.tile([1, 1], fp32, tag="out_sb")
        nc.vector.tensor_copy(out_sb[:], red[:])
        nc.sync.dma_start(out_dram, out_sb[:])

    nc.compile()
    return nc


def make_in_maps(query: np.ndarray, target: np.ndarray):
    """Host-side sharding + pair-packing metadata (per-core rolled copies)."""
    query = np.ascontiguousarray(np.asarray(query), dtype=np.float32)
    tgt = np.asarray(target).reshape(-1)

    # balance rows across cores by positive-pair count (any assignment is
    # valid: each core sees a full permuted copy with its rows first)
    npos_all = np.array([np.sum(tgt == tgt[i]) - 1 for i in range(N)])
    ncnt = int(np.sum(npos_all > 0))
    loads = [0] * NCORES
    assign = [[] for _ in range(NCORES)]
    for i in sorted(range(N), key=lambda i: -npos_all[i]):
        cands = [c for c in range(NCORES) if len(assign[c]) < RPC]
        c = min(cands, key=lambda c: loads[c])
        assign[c].append(i)
        loads[c] += int(npos_all[i])

    cores = []
    for c in range(NCORES):
        mine = assign[c]
        others = [i for i in range(N) if i not in set(mine)]
        perm = np.array(mine + others)
        t_r = tgt[perm]
        rows = []  # per row: positive indices (in permuted coords)
        for q in range(RPC):
            pos = np.flatnonzero(t_r == t_r[q])
            pos = pos[pos != q]
            assert len(pos) <= SLOTS, f"npos {len(pos)} > SLOTS {SLOTS}"
            rows.append(pos)
        # bin-pack rows (row-atomic, best-fit decreasing) into <=128-pair bins
        blocks = []
        fill = []
        order = sorted((q for q in range(RPC) if len(rows[q]) > 0),
                       key=lambda q: -len(rows[q]))
        for q in order:
            npos = len(rows[q])
            best = -1
            for i, f in enumerate(fill):
                if f + npos <= 128 and (best < 0 or f > fill[best]):
                    best = i
            if best < 0:
                blocks.append([q])
                fill.append(npos)
            else:
                blocks[best].append(q)
                fill[best] += npos
        cores.append((perm, rows, blocks))
    nblk = max(len(b) for _, _, b in cores)

    iota_host = np.ascontiguousarray(
        np.broadcast_to(np.arange(N, dtype=np.float32), (128, N)))
    in_maps = []
    for perm, rows, blocks in cores:
        q_r = np.ascontiguousarray(query[perm])
        sel = np.full((128, nblk), -1.0, dtype=np.float32)
        w = np.zeros((128, nblk), dtype=np.float32)
        maskg = np.zeros((128, SLOTS * nblk), dtype=np.float32)
        rep = np.zeros((RPC, 128 * nblk), dtype=np.float32)
        bdgs = np.zeros((128, 128 * nblk), dtype=np.float32)
        ibs = np.zeros((128, SLOTS * nblk), dtype=np.float32)
        for b, rowlist in enumerate(blocks):
            p = 0
            for q in rowlist:
                npos = len(rows[q])
                pr = range(p, p + npos)
                for s, j in enumerate(rows[q]):
                    sel[p + s, b] = float(j)
                    w[p + s, b] = 1.0 / npos
                    ibs[p + s, SLOTS * b + s] = 1.0
                    maskg[p + s, SLOTS * b:SLOTS * b + npos] = 1.0
                for k in pr:
                    for p2 in pr:
                        bdgs[k, 128 * b + p2] = 1.0
                    rep[q, 128 * b + k] = 1.0
                p += npos
        in_maps.append({
            "q": q_r,
            "qt": np.ascontiguousarray(q_r.T),
            "iota": iota_host,
            "sel": sel, "w": w, "maskg": maskg,
            "rep": rep, "bdgs": bdgs, "ibs": ibs,
        })
    return in_maps, nblk, ncnt


_NC_CACHE = {}


def kernel(query: np.ndarray, target: np.ndarray) -> np.ndarray:
    from concourse import bass_utils

    in_maps, nblk, ncnt = make_in_maps(query, target)
    global _NC_CACHE
    if nblk not in _NC_CACHE:
        _NC_CACHE[nblk] = _build_program(nblk)
    nc = _NC_CACHE[nblk]

    res = bass_utils.run_bass_kernel_spmd(nc, in_maps, core_ids=list(range(NCORES)))
    num = 0.0
    for c in range(NCORES):
        num += float(res.results[c]["out"].reshape(-1)[0])
    mean_ap = num / max(float(ncnt), 1.0)
    return np.float32(1.0 - mean_ap)

